# revision 38
# baseline (speedup 1.0000x reference)
"""AlphaFold-style gated attention (pair bias + sigmoid gating) on 8 Trainium2
NeuronCores.

Problem shapes (hardcoded): B=4, Q=K=1024, C=256, H=8, D=32, fp32.

Sharding: (batch x head-group) -> core = b*2 + hg; each core handles 1 batch
and 4 heads.  Each core computes a partial output [Q, C] (its 4 heads pushed
through the output projection); the host sums the two partials (+ bo) per
batch.

Per-core device kernel, fp16 matmul datapath (fp32 PSUM accumulation):
  qT/kT    = W @ x.T                [HD=128, Q]  (q scale folded into Wq)
  eg       = exp(-(Wg x + bg))      ACT Exp; the sigmoid gate 1/(1+eg) is
                                    folded into the rowsum reciprocal
  v        = kv_x @ Wv.T            [K-tile, HD] x 4 paired tiles
  S.T      = k_h^T-block @ q_T      [K-tile, Q]  per head, PSUM fp32
  es       = exp(S.T)               ACT, zero bias, fp16 out
  e2       = es * ep                DVE f16 2x mode; ep = exp(pair + mask
                                    - SHIFT) precomputed on host (softmax
                                    factorization exp(S+p) = exp(S)exp(p))
  o.T     += v_h.T @ e2             col-packed 4 heads -> [128, Q] PSUM
  rowsum  += ones @ e2              col-packed, M=32 dup rows
  o_eff    = o.T * recip((1+eg.T)*rowsum)   DVE, fp16 out
  out      = o_eff.T @ Wo.T         natural [Q, C], fp16 matmul; bo added
                                    on the host during the gather

The exp SHIFT keeps es/e2 within fp16 range; it cancels in o/rowsum.

Performance notes (measured on trn2 via axon):
- Engine queues execute in scheduled (~program) order; the emission order
  IS the software pipeline.  QK/exp/mul run one unit ahead of AV/rowsum,
  sweep 0's normalize/output-projection is spread through sweep 1's units.
- The pair identity-matmul accumulation of the original version (64 extra
  128x128x512 matmuls, ~20us PE) became a host exp() + DVE multiply.
- Exp is the ONLY ACT function used (sigmoid via exp + reciprocal fold),
  so the single-slot ACT table cache never reloads mid-run; the table is
  preloaded with a dummy during the DMA window.
- ACT is the pacing engine: 32 exps x (1024+352)cyc/1.2GHz ~ 34us.  PSUM
  (8 banks: 3x2 S tiles + o/r accumulators) limits deeper exp batching.
- Every dma_start costs ~0.6us of ring time and its completion semaphore
  fires ~4.5us after the transfer ends, so small tensors are packed into
  3 transfers, inputs ride the otherwise-idle scalar ring, and outputs
  alternate rings.  gpsimd is unused (no SWDGE, cheap drains, and it
  cannot touch PSUM anyway).
"""

import math

import numpy as np

B, Q, K, C, H, D = 4, 1024, 1024, 256, 8, 32
HPG = 4  # heads per group
HG = 2  # head groups
NCORES = 8
KT = K // 128  # 8 K-tiles
SHIFT = 2.0  # exp shift (softmax-invariant), keeps es*ep < fp16 max

PAIR_BUFS = 32
ES_BUFS = 8
E2_BUFS = 12
LAG = 1  # av_rowsum lag behind qk_exp_mul, in units


def _build_program():
    import concourse.bass as bass
    import concourse.tile as tile
    from concourse import bacc, mybir

    f32 = mybir.dt.float32
    f16 = mybir.dt.float16
    AF = mybir.ActivationFunctionType
    AL = mybir.AluOpType

    nc = bacc.Bacc("TRN2", target_bir_lowering=False, debug=False)

    # ---- I/O (host-prepped layouts, see _shard_inputs) ----------------
    # Every dma_start costs ~0.6us on the HWDGE ring regardless of size, so
    # the small tensors are packed into three transfers:
    #   w4   f16 [128, 1024] = wq | wk | wv | wg
    #   sm16 f16 [128, 288]  = ones | wo
    #   sm32 f32 [128, 1]    = -bg
    d_qx = nc.dram_tensor("qx", [128, 2 * Q], f16, kind="ExternalInput").ap()
    d_kvx = nc.dram_tensor("kvx", [128, 2 * K], f16, kind="ExternalInput").ap()
    d_ep = nc.dram_tensor("ep", [2, KT, 2, 128, Q], f16, kind="ExternalInput").ap()
    d_w4 = nc.dram_tensor("w4", [128, 1024], f16, kind="ExternalInput").ap()
    d_sm16 = nc.dram_tensor("sm16", [128, 288], f16, kind="ExternalInput").ap()
    d_sm32 = nc.dram_tensor("sm32", [128, 1], f32, kind="ExternalInput").ap()
    d_out = nc.dram_tensor("out", [Q, C], f32, kind="ExternalOutput").ap()

    with tile.TileContext(nc) as tc:
        from contextlib import ExitStack

        with ExitStack() as ctx:
            cp = ctx.enter_context(tc.tile_pool(name="consts", bufs=1))
            act_p = ctx.enter_context(tc.tile_pool(name="acts", bufs=1))
            pair_p = ctx.enter_context(tc.tile_pool(name="pair", bufs=PAIR_BUFS))
            es_p = ctx.enter_context(tc.tile_pool(name="es", bufs=ES_BUFS))
            e2_p = ctx.enter_context(tc.tile_pool(name="e2", bufs=E2_BUFS))
            nrm_p = ctx.enter_context(tc.tile_pool(name="nrm", bufs=4))
            mid_p = ctx.enter_context(tc.tile_pool(name="mid", bufs=1))
            out_p = ctx.enter_context(tc.tile_pool(name="outs", bufs=3))

            w4 = cp.tile([128, 1024], f16)
            sm16 = cp.tile([128, 288], f16)
            sm32 = cp.tile([128, 1], f32)
            wq = w4[:, 0:256]
            wk = w4[:, 256:512]
            wv = w4[:, 512:768]
            wg = w4[:, 768:1024]
            ones = sm16[:, 0:32]
            wo = sm16[:, 32:288]
            nbg = sm32[:, 0:1]  # NEGATED bg (host-side)
            qx = act_p.tile([128, 2 * Q], f16)
            kvx = act_p.tile([128, 2 * K], f16)

            # input DMAs on the scalar HWDGE ring: the sync ring streams the
            # 8MB of exp(pair) tiles back-to-back, and completion receipts
            # there lag several transfers behind; the near-idle scalar ring
            # signals completion promptly so phase 1 can start early.
            nc.scalar.dma_start(w4[:], d_w4[:])
            nc.scalar.dma_start(qx[:], d_qx[:])
            nc.scalar.dma_start(kvx[:], d_kvx[:])
            nc.scalar.dma_start(sm16[:], d_sm16[:])
            nc.scalar.dma_start(sm32[:], d_sm32[:])

            # preload the ACT Exp table off the critical path: a dummy
            # activation on a memset scratch during the DMA window.  Exp is
            # the ONLY activation function the kernel uses (the sigmoid gate
            # is computed as e=exp(-(z+bg)) with 1/(1+e) folded into the
            # rowsum reciprocal), so the single-slot ACT table cache never
            # reloads mid-run.
            zeros = cp.tile([128, 1], f32)
            scr1 = cp.tile([128, 1], f32)
            nc.vector.memset(zeros[:], 0.0)
            nc.scalar.activation(scr1[:], zeros[:], AF.Exp)
            zeros = zeros[:]

            # issue all exp(pair) DMAs up-front (pool slots throttle them in
            # order); tile (qh, kc, hp2) = heads (2*hp2, 2*hp2+1) side by
            # side for q-half qh -- matches the es tile layout exactly.
            pair_t = {}
            for kc in range(KT):
                for qh in range(2):
                    for hp2 in range(2):
                        t = pair_p.tile(
                            [128, Q], f16, tag="pair", name=f"ep_{qh}_{kc}_{hp2}"
                        )
                        nc.sync.dma_start(t[:], d_ep[qh, kc, hp2])
                        pair_t[(qh, kc, hp2)] = t

            q_sb = mid_p.tile([128, Q], f16)
            k_sb = mid_p.tile([128, K], f16)
            g_sb = mid_p.tile([128, Q], f32)
            v_sb = [
                mid_p.tile([128, 256], f16, tag=f"v{i}", name=f"v{i}")
                for i in range(KT // 2)
            ]

            # ---- phases 1-4: projections + attention, one pipeline -------
            # All 32 exp(pair) chunks stay resident in SBUF.  Engine queues
            # execute in scheduled (~program) order, so emission order is
            # the software pipeline: QK/exp/mul of unit u are emitted one
            # step ahead of AV/rowsum of unit u-1, keeping ACT fed; sweep
            # 0's normalization/output-projection is deferred into sweep
            # 1's early units so its DVE/PE work hides under sweep-1
            # compute instead of stalling the chain.  The projections are
            # threaded through the same scope (q/k + v through the ps_o
            # slot, gate through the ps_r slot, both released before the
            # o/r accumulators claim them) so the first attention units are
            # emitted right after the q0/k0 projections and the exp stream
            # starts as early as possible.
            # PSUM: 3 S tiles (6 banks) + (o,r) accumulators (2 banks) = 8.
            with (
                tc.tile_pool(name="ps_s", bufs=3, space="PSUM") as ps_s,
                tc.tile_pool(name="ps_o", bufs=1, space="PSUM") as ps_o,
                tc.tile_pool(name="ps_r", bufs=1, space="PSUM") as ps_r,
            ):
                o_eff = mid_p.tile([128, Q], f16)
                o_ps = {}
                r_ps = {}
                e2_t = {}

                def proj_q(qh):
                    ps = ps_o.tile([128, 512], f32, tag="o", name="ps_q")
                    for j in range(2):
                        nc.tensor.matmul(
                            ps[:],
                            wq[:, bass.ts(j, 128)],
                            qx[:, j * Q + qh * 512 :][:, :512],
                            start=(j == 0),
                            stop=(j == 1),
                        )
                    nc.vector.tensor_copy(q_sb[:, bass.ts(qh, 512)], ps[:])

                def proj_k(qh):
                    ps = ps_r.tile([128, 512], f32, tag="r", name="ps_k")
                    for j in range(2):
                        nc.tensor.matmul(
                            ps[:],
                            wk[:, bass.ts(j, 128)],
                            kvx[:, j * K + qh * 512 :][:, :512],
                            start=(j == 0),
                            stop=(j == 1),
                        )
                    nc.vector.tensor_copy(k_sb[:, bass.ts(qh, 512)], ps[:])

                def proj_vpair(p):  # k-tiles 2p, 2p+1 -> v_sb[p]
                    ps = ps_o.tile([128, 256], f32, tag="o", name="ps_v")
                    for half in range(2):
                        kt = 2 * p + half
                        for j in range(2):
                            nc.tensor.matmul(
                                ps[:, bass.ts(half, 128)],
                                kvx[:, j * K + kt * 128 :][:, :128],
                                wv[:, bass.ts(j, 128)],
                                start=(j == 0),
                                stop=(j == 1),
                                skip_group_check=True,
                            )
                    nc.vector.tensor_copy(v_sb[p][:], ps[:])

                def proj_g(qh):  # g_sb holds e = exp(-(Wg x + bg))
                    ps = ps_r.tile([128, 512], f32, tag="r", name="ps_g")
                    for j in range(2):
                        nc.tensor.matmul(
                            ps[:],
                            wg[:, bass.ts(j, 128)],
                            qx[:, j * Q + qh * 512 :][:, :512],
                            start=(j == 0),
                            stop=(j == 1),
                        )
                    nc.scalar.activation(
                        g_sb[:, bass.ts(qh, 512)], ps[:], AF.Exp,
                        bias=nbg, scale=-1.0,
                    )

                def qk_exp_mul(qh, kc):
                    sp = [
                        ps_s.tile(
                            [128, 1024], f32, tag="s", name=f"sp_{kc}_{qh}_{hp2}"
                        )
                        for hp2 in range(2)
                    ]
                    for h in range(HPG):
                        hp = slice(32 * h, 32 * h + 32)
                        nc.tensor.matmul(
                            sp[h // 2][:, bass.ts(h % 2, 512)],
                            k_sb[hp, bass.ts(kc, 128)],
                            q_sb[hp, bass.ts(qh, 512)],
                            start=True,
                            stop=True,
                            tile_position=(32 * h, 0),
                            skip_group_check=True,
                        )
                    e2s = []
                    for hp2 in range(2):
                        es = es_p.tile(
                            [128, 1024], f16, tag="es", name=f"es_{kc}_{qh}_{hp2}"
                        )
                        nc.scalar.activation(es[:], sp[hp2][:], AF.Exp, bias=zeros)
                        e2 = e2_p.tile(
                            [128, 1024], f16, tag="e2", name=f"e2_{kc}_{qh}_{hp2}"
                        )
                        nc.vector.tensor_mul(e2[:], es[:], pair_t[(qh, kc, hp2)][:])
                        e2s.append(e2)
                    e2_t[(qh, kc)] = e2s

                def av_rowsum(qh, kc):
                    if kc == 0:
                        o_ps[qh] = ps_o.tile(
                            [128, 512], f32, tag="o", name=f"o_ps{qh}"
                        )
                        r_ps[qh] = ps_r.tile(
                            [128, 512], f32, tag="r", name=f"r_ps{qh}"
                        )
                    e2s = e2_t.pop((qh, kc))
                    vb = 128 * (kc % 2)
                    for h in range(HPG):
                        hp = slice(32 * h, 32 * h + 32)
                        nc.tensor.matmul(
                            o_ps[qh][hp, :],
                            v_sb[kc // 2][:, vb + 32 * h : vb + 32 * h + 32],
                            e2s[h // 2][:, bass.ts(h % 2, 512)],
                            start=(kc == 0),
                            stop=(kc == KT - 1),
                            tile_position=(0, 32 * h),
                            skip_group_check=True,
                        )
                    for h in range(HPG):
                        hp = slice(32 * h, 32 * h + 32)
                        nc.tensor.matmul(
                            r_ps[qh][hp, :],
                            ones[:],
                            e2s[h // 2][:, bass.ts(h % 2, 512)],
                            start=(kc == 0),
                            stop=(kc == KT - 1),
                            tile_position=(0, 32 * h),
                            skip_group_check=True,
                        )

                def normalize(qh):
                    # o_eff = o * sigmoid(z) / r = o * recip((1+e)*r)
                    den = nrm_p.tile([128, 512], f32, tag="den", name="den")
                    nc.vector.scalar_tensor_tensor(
                        den[:], g_sb[:, bass.ts(qh, 512)], 1.0, r_ps[qh][:],
                        op0=AL.add, op1=AL.mult,
                    )
                    recip = nrm_p.tile([128, 512], f32, tag="recip", name="recip")
                    nc.vector.reciprocal_approx_fast(recip[:], den[:])
                    nc.vector.tensor_mul(
                        o_eff[:, bass.ts(qh, 512)], o_ps[qh][:], recip[:]
                    )

                def outproj_qt(qt):
                    # bo is added on the host during the partial-sum gather;
                    # the PSUM->SBUF move is a single-src DVE copy (faster
                    # mode than tensor_tensor) and the out DMAs alternate
                    # between the two HWDGE rings to shorten the tail.
                    ps = ps_s.tile([128, 256], f32, tag="s", name="ps_out")
                    nc.tensor.matmul(
                        ps[:],
                        o_eff[:, bass.ts(qt, 128)],
                        wo[:],
                        start=True,
                        stop=True,
                    )
                    ot = out_p.tile([128, 256], f32, tag="ot", name="ot")
                    nc.vector.tensor_copy(ot[:], ps[:])
                    ring = nc.sync if qt % 2 == 0 else nc.scalar
                    ring.dma_start(d_out[bass.ts(qt, 128), :], ot[:])

                # Emission schedule.  The projections ride through the ps_o
                # and ps_r slots and are fully emitted before av_rowsum(0,0)
                # claims those slots for the o/r accumulators.  AV/rowsum
                # lags QK/exp/mul by one unit so its DVE input (the e2
                # multiply) is always ready when the in-order PE queue
                # reaches it; otherwise a bubble recirculates through the
                # QK->exp->mul->AV loop.
                proj_q(0)
                proj_k(0)
                qk_exp_mul(0, 0)
                proj_q(1)
                proj_k(1)
                qk_exp_mul(0, 1)
                for p in range(KT // 2):
                    proj_vpair(p)
                proj_g(0)
                proj_g(1)
                if LAG == 1:
                    av_rowsum(0, 0)
                for kc in range(2, KT):
                    qk_exp_mul(0, kc)
                    av_rowsum(0, kc - LAG)
                for kc in range(KT):
                    qk_exp_mul(1, kc)
                    pu = 8 + kc - LAG  # global index of lagged unit
                    if pu < 8:
                        av_rowsum(0, pu)
                    elif pu == 8:
                        normalize(0)  # frees o/r banks before av_rowsum(1,0)
                        av_rowsum(1, 0)
                    else:
                        if 3 <= pu - 8 + 2 < 7:
                            outproj_qt(pu - 8 - 1)  # spread sweep-0 outproj
                        av_rowsum(1, pu - 8)
                for kc in range(KT - LAG, KT):
                    av_rowsum(1, kc)
                normalize(1)
                for qt in range(4, 8):
                    outproj_qt(qt)

    nc.compile()
    return nc


_NC_CACHE = None


def _get_program():
    global _NC_CACHE
    if _NC_CACHE is None:
        _NC_CACHE = _build_program()
    return _NC_CACHE


def _shard_inputs(q_x, kv_x, bias_mask, bias_pair, Wq, Wk, Wv, Wo, bo, Wg, bg):
    """Build the 8 per-core input maps."""
    f = np.float32
    f16 = np.float16
    scale = 1.0 / math.sqrt(D)

    def fold2(w_t):  # [256, M] -> [128, 2*M] sbuf layout
        return np.ascontiguousarray(
            w_t.reshape(2, 128, w_t.shape[1]).transpose(1, 0, 2).reshape(128, -1)
        )

    in_maps = []
    for core in range(NCORES):
        b, hg = core // HG, core % HG
        hs = slice(hg * 128, hg * 128 + 128)  # H*D slice for this head group
        qxT = np.ascontiguousarray(q_x[b].T).astype(f)  # [256, 1024]
        kvxT = np.ascontiguousarray(kv_x[b].T).astype(f)
        # exp(pair) tiles: [qh, kc, hp2, 128, Q]; tile (qh,kc,hp2) holds heads
        # (2*hp2, 2*hp2+1) side by side for q-half qh, k-tile kc (transposed
        # to [k, q] to match the S.T layout).
        epT = np.exp(
            bias_pair[b, hg * HPG : hg * HPG + HPG].astype(f)
            + bias_mask[b, 0, 0].astype(f)[None, None, :]
            - SHIFT
        ).transpose(0, 2, 1)  # [4, K, Q]
        ep = (
            epT.reshape(HPG, KT, 128, 2, 512)
            .transpose(3, 1, 0, 2, 4)  # [qh, kc, h, 128, 512]
            .reshape(2, KT, 2, 2, 128, 512)
            .transpose(0, 1, 2, 4, 3, 5)  # [qh, kc, hp2, 128, hi, 512]
            .reshape(2, KT, 2, 128, Q)
        )
        w4 = np.concatenate(
            [
                fold2(np.ascontiguousarray(Wq[hs].T) * scale),
                fold2(np.ascontiguousarray(Wk[hs].T)),
                fold2(np.ascontiguousarray(Wv[hs].T)),
                fold2(np.ascontiguousarray(Wg[hs].T)),
            ],
            axis=1,
        )
        sm16 = np.concatenate(
            [np.ones((128, 32), f), np.ascontiguousarray(Wo[:, hs].T)], axis=1
        )
        sm32 = np.ascontiguousarray(-bg[hs].reshape(128, 1)).astype(f)
        m16 = {
            "qx": fold2(qxT),
            "kvx": fold2(kvxT),
            "ep": ep,
            "w4": w4,
            "sm16": sm16,
        }
        m = {k: np.ascontiguousarray(v, f16) for k, v in m16.items()}
        m["sm32"] = np.ascontiguousarray(sm32, f)
        in_maps.append(m)
    return in_maps


def run_on_cores(in_maps, trace=False, trace_kwargs={}):
    from concourse.bass_utils import run_bass_kernel_spmd

    nc = _get_program()
    return run_bass_kernel_spmd(
        nc, in_maps, list(range(NCORES)), trace=trace, trace_kwargs=trace_kwargs
    )


def kernel(q_x, kv_x, bias_mask, bias_pair, Wq, Wk, Wv, Wo, bo, Wg, bg):
    in_maps = _shard_inputs(
        q_x, kv_x, bias_mask, bias_pair, Wq, Wk, Wv, Wo, bo, Wg, bg
    )
    res = run_on_cores(in_maps).results
    out = np.empty((B, Q, C), np.float32)
    bo32 = bo.astype(np.float32)
    for b in range(B):
        out[b] = res[b * HG + 0]["out"] + res[b * HG + 1]["out"] + bo32
    return out
